# revision 1
# baseline (speedup 1.0000x reference)
"""Trainium2 Bass kernel for CausalSelfAttention with KV-prefix cache.

Problem (hardcoded): B=2, T=2048, C=1024, H=16, D=64, P=2048.
Sharding: 8 cores = 2 (batch) x 4 (head groups of 4 heads).
Each core computes, for its (b, 4 heads):
  qkv slice -> prefix+causal softmax -> AV -> partial W_proj product.
Host sums the 4 per-core partial projections per batch and transposes.

All attention math runs in a transposed layout (features/keys on the
partition dim) so no on-device transposes are ever needed:
  x^T [C,T] -> q^T,k^T (pair-packed [128,T]) via lhsT=W_attn slice
  S^T [keys, tq] via row-tiled (2 heads) K=64 matmuls
  exp on ScalarE (PSUM->SBUF, bf16, fused scale=1/sqrt(D))
  AV + denom via col-tiled matmuls accumulated in PSUM over key chunks
  y^T / denom via gpsimd partition-broadcast + DVE multiply
  out^T = W_proj_slice^T @ y^T  (per-core partial)
"""

import numpy as np
import ml_dtypes
from contextlib import ExitStack

import concourse.bacc as bacc
import concourse.tile as tile
import concourse.mybir as mybir
from concourse.bass_utils import run_bass_kernel_spmd

F32 = mybir.dt.float32
F32R = mybir.dt.float32r
BF16 = mybir.dt.bfloat16
EXP = mybir.ActivationFunctionType.Exp
COPY = mybir.ActivationFunctionType.Copy
MULT = mybir.AluOpType.mult

B, T, C, H, D, P = 2, 2048, 1024, 16, 64, 2048
HPC = 4            # heads per core
NPAIR = 2          # head pairs per core
TQ = 512           # query block (matmul free dim)
KC = 128           # key chunk (PSUM partition dim)
G = 3              # key chunks per exp group (6 PSUM banks with 2 heads)
SCALE = 1.0 / np.sqrt(D)


def build_kernel(t=T, p=P, c=C, n_cores=8, dbg=False, phases=(1, 2, 3), p2sub=7):
    """Build + compile the SPMD Bass program. Same program on every core."""
    nt = t // TQ            # query blocks
    npc = p // KC           # prefix key chunks
    nck = c // 128          # C contraction chunks
    ntc = t // 128          # T rows in 128-chunks

    nc = bacc.Bacc("TRN2", target_bir_lowering=False, debug=False,
                   num_devices=n_cores)
    dbg_t = {}
    if dbg:
        for nm, shp in [("dbg_qT", [128, 512]), ("dbg_kT", [128, 512]),
                        ("dbg_vt", [128, 256]), ("dbg_eb", [128, 2 * G, TQ]),
                        ("dbg_db", [128, TQ]), ("dbg_bc", [128, TQ]),
                        ("dbg_y", [128, TQ]), ("dbg_yu", [128, TQ])]:
            dt_ = BF16 if nm in ("dbg_eb", "dbg_vt") else F32
            dbg_t[nm] = nc.dram_tensor(nm, shp, dt_, kind="ExternalOutput").ap()

    x_t = nc.dram_tensor("x_t", [c, t], F32, kind="ExternalInput").ap()
    w_qk = nc.dram_tensor("w_qk", [c, 2 * HPC * D], F32, kind="ExternalInput").ap()
    w_v = nc.dram_tensor("w_v", [c, HPC * D], F32, kind="ExternalInput").ap()
    ckt = nc.dram_tensor("ckt", [NPAIR, 128, p], F32, kind="ExternalInput").ap()
    cv = nc.dram_tensor("cv", [NPAIR, 128, npc, 128], BF16, kind="ExternalInput").ap()
    wp = nc.dram_tensor("wp", [NPAIR, 128, c], F32, kind="ExternalInput").ap()
    masks = nc.dram_tensor("masks", [128, 4, TQ], BF16, kind="ExternalInput").ap()
    ones = nc.dram_tensor("ones", [128, 1], BF16, kind="ExternalInput").ap()
    bsel = nc.dram_tensor("bsel", [33, 128], F32, kind="ExternalInput").ap()
    zrd = nc.dram_tensor("zrd", [33, TQ], F32, kind="ExternalInput").ap()
    out_t = nc.dram_tensor("out_t", [c, t], F32, kind="ExternalOutput").ap()

    with tile.TileContext(nc) as tc, ExitStack() as top:
        const = top.enter_context(tc.tile_pool(name="const", bufs=1))
        persist = top.enter_context(tc.tile_pool(name="persist", bufs=1))

        # ---- persistent SBUF tensors -------------------------------------
        qT = [persist.tile([128, t], F32R, tag=f"qT{i}", name=f"qT{i}") for i in range(NPAIR)]
        kT = [persist.tile([128, t], F32R, tag=f"kT{i}", name=f"kT{i}") for i in range(NPAIR)]
        cktT = [persist.tile([128, p], F32R, tag=f"cktT{i}", name=f"cktT{i}") for i in range(NPAIR)]
        vt = persist.tile([128, ntc, 2 * 128], BF16, tag="vt", name="vt")
        cvt = [persist.tile([128, npc, 128], BF16, tag=f"cvt{i}", name=f"cvt{i}") for i in range(NPAIR)]
        wpt = [persist.tile([128, c], F32R, tag=f"wpt{i}", name=f"wpt{i}") for i in range(NPAIR)]
        maskt = const.tile([128, 4, TQ], BF16, tag="maskt", name="maskt")
        onest = const.tile([128, 1], BF16, tag="onest", name="onest")
        bselt = const.tile([33, 128], F32R, tag="bselt", name="bselt")
        ysb = [persist.tile([128, nt, TQ], F32R, tag=f"ysb{i}", name=f"ysb{i}") for i in range(NPAIR)]
        rd = persist.tile([33, TQ], F32R, tag="rd", name="rd")
        nc.sync.dma_start(rd[:], zrd[:, :].bitcast(F32R))

        nc.sync.dma_start(maskt[:], masks[:, :, :])
        nc.sync.dma_start(onest[:], ones[:, :])
        nc.sync.dma_start(bselt[:], bsel[:, :].bitcast(F32R))
        for i in range(NPAIR):
            nc.sync.dma_start(cktT[i][:], ckt[i, :, :].bitcast(F32R))
            nc.sync.dma_start(cvt[i][:], cv[i, :, :, :])
            nc.sync.dma_start(wpt[i][:], wp[i, :, :].bitcast(F32R))

        # ---- phase 1: QKV projection -------------------------------------
        with ExitStack() as ph1:
          if 1 in phases:
              qkv_in = ph1.enter_context(tc.tile_pool(name="qkv_in", bufs=1))
              ps_qk = ph1.enter_context(tc.tile_pool(name="ps_qk", bufs=4, space="PSUM"))
              ps_v = ph1.enter_context(tc.tile_pool(name="ps_v", bufs=4, space="PSUM"))

              xt = qkv_in.tile([128, nck, t], F32R, tag="xt", name="xt")
              wqkt = qkv_in.tile([128, nck, 2 * HPC * D], F32R, tag="wqkt", name="wqkt")
              wvt = qkv_in.tile([128, nck, HPC * D], F32R, tag="wvt", name="wvt")
              for kc_ in range(nck):
                  nc.sync.dma_start(xt[:, kc_, :], x_t[kc_ * 128:(kc_ + 1) * 128, :].bitcast(F32R))
                  nc.sync.dma_start(wqkt[:, kc_, :], w_qk[kc_ * 128:(kc_ + 1) * 128, :].bitcast(F32R))
                  nc.sync.dma_start(wvt[:, kc_, :], w_v[kc_ * 128:(kc_ + 1) * 128, :].bitcast(F32R))

              # q^T / k^T: out chunk mc (128 rows = one head pair of q or k)
              for mc in range(4):
                  dest = qT[mc] if mc < 2 else kT[mc - 2]
                  for nb in range(nt):
                      ps = ps_qk.tile([128, TQ], F32, tag="ps_qk", name="psqk")
                      for kc_ in range(nck):
                          nc.tensor.matmul(
                              ps[:],
                              wqkt[:, kc_, mc * 128:(mc + 1) * 128],
                              xt[:, kc_, nb * TQ:(nb + 1) * TQ],
                              start=(kc_ == 0), stop=(kc_ == nck - 1),
                          )
                      nc.scalar.activation(dest[:, nb * TQ:(nb + 1) * TQ], ps[:], COPY)

              # v in natural layout [t, 256]
              for tc_ in range(ntc):
                  ps = ps_v.tile([128, HPC * D], F32, tag="ps_v", name="psv")
                  for kc_ in range(nck):
                      nc.tensor.matmul(
                          ps[:],
                          xt[:, kc_, tc_ * 128:(tc_ + 1) * 128],
                          wvt[:, kc_, :],
                          start=(kc_ == 0), stop=(kc_ == nck - 1),
                      )
                  nc.scalar.activation(vt[:, tc_, :], ps[:], COPY)

        if dbg:
            nc.sync.dma_start(dbg_t["dbg_qT"][:, :], qT[0][:, 0:512].bitcast(F32))
            nc.sync.dma_start(dbg_t["dbg_kT"][:, :], kT[0][:, 0:512].bitcast(F32))
            nc.sync.dma_start(dbg_t["dbg_vt"][:, :], vt[:, 0, :])

        # ---- phase 2: attention ------------------------------------------
        with ExitStack() as ph2:
          if 2 in phases:
              ps_s = ph2.enter_context(tc.tile_pool(name="ps_s", bufs=1, space="PSUM"))
              ps_y = ph2.enter_context(tc.tile_pool(name="ps_y", bufs=1, space="PSUM"))
              ps_d = ph2.enter_context(tc.tile_pool(name="ps_d", bufs=1, space="PSUM"))
              expp = ph2.enter_context(tc.tile_pool(name="expp", bufs=2))
              nrm = ph2.enter_context(tc.tile_pool(name="nrm", bufs=2))

              for pair in range(NPAIR):
                  for tb in range(nt):
                      ncur = (tb + 1) * (TQ // KC)   # current-key chunks
                      nkc = npc + ncur               # total key chunks
                      yb = ps_y.tile([128, TQ], F32, tag="yb", name="yb")
                      db = ps_d.tile([128, TQ], F32, tag="db", name="db")
                      first, last = 0, nkc - 1
                      g0 = 0
                      while g0 < nkc:
                          gc = min(G, nkc - g0)
                          sb = ps_s.tile([128, 2 * G, TQ], F32, tag="sb", name="sb")
                          # scores S^T for both heads (row-tiled K=64 pairs)
                          for i in range(gc):
                              kc_ = g0 + i
                              if kc_ < npc:
                                  ksrc = cktT[pair]
                                  klo = kc_ * KC
                              else:
                                  ksrc = kT[pair]
                                  klo = (kc_ - npc) * KC
                              for h in range(2):
                                  nc.tensor.matmul(
                                      sb[:, h * G + i, :],
                                      ksrc[h * 64:(h + 1) * 64, klo:klo + KC],
                                      qT[pair][h * 64:(h + 1) * 64, tb * TQ:(tb + 1) * TQ],
                                      start=True, stop=True,
                                      tile_position=(h * 64, 0),
                                      skip_group_check=True,
                                  )
                          # fused exp over the group's banks -> bf16 SBUF
                          eb = expp.tile([128, 2 * G, TQ], BF16, tag="eb", name="eb")
                          if gc == G:
                              nc.scalar.activation(eb[:], sb[:], EXP, scale=SCALE)
                          else:
                              for h in range(2):
                                  nc.scalar.activation(
                                      eb[:, h * G:h * G + gc, :],
                                      sb[:, h * G:h * G + gc, :],
                                      EXP, scale=SCALE)
                          # causal masks on diagonal chunks
                          for i in range(gc if p2sub >= 2 else 0):
                              kc_ = g0 + i
                              j = kc_ - (nkc - 4)
                              if j >= 0:
                                  for h in range(2):
                                      nc.vector.tensor_tensor(
                                          eb[:, h * G + i, :], eb[:, h * G + i, :],
                                          maskt[:, j, :], MULT)
                          # AV (col-tiled pair) + denominators, PSUM-accumulated
                          for i in range(gc if p2sub >= 3 else 0):
                              kc_ = g0 + i
                              if kc_ < npc:
                                  vsrc_e = cvt[pair][:, kc_, 0:64]
                                  vsrc_o = cvt[pair][:, kc_, 64:128]
                              else:
                                  ck = kc_ - npc
                                  vsrc_e = vt[:, ck, pair * 128:pair * 128 + 64]
                                  vsrc_o = vt[:, ck, pair * 128 + 64:pair * 128 + 128]
                              st = (kc_ == first)
                              sp = (kc_ == last)
                              nc.tensor.matmul(yb[0:64, :], vsrc_e, eb[:, i, :],
                                               start=st, stop=sp,
                                               tile_position=(0, 0),
                                               skip_group_check=True)
                              nc.tensor.matmul(yb[64:128, :], vsrc_o, eb[:, G + i, :],
                                               start=st, stop=sp,
                                               tile_position=(0, 64),
                                               skip_group_check=True)
                              if p2sub < 4:
                                  continue
                              nc.tensor.matmul(db[0:1, :], onest[:], eb[:, i, :],
                                               start=st, stop=sp,
                                               tile_position=(0, 0),
                                               skip_group_check=True)
                              nc.tensor.matmul(db[32:33, :], onest[:], eb[:, G + i, :],
                                               start=st, stop=sp,
                                               tile_position=(0, 32),
                                               skip_group_check=True)
                          if dbg and pair == 0 and tb == 0 and g0 == 0:
                              nc.sync.dma_start(dbg_t["dbg_eb"][:, :, :], eb[:])
                          g0 += gc

                      # evacuate unnormalized y^T (releases the PSUM bank fast)
                      if p2sub < 3:
                          continue
                      ysl = ysb[pair][:, tb, :]
                      nc.vector.tensor_copy(ysl, yb[:])
                      # reciprocal of denominators, broadcast via DRAM bounce
                      if p2sub < 5:
                          continue
                      with nc.allow_low_precision(reason="recip->f32r for bcast mm"):
                          nc.vector.reciprocal(rd[0:1, :], db[0:1, :])
                          nc.vector.reciprocal(rd[32:33, :], db[32:33, :])
                      # broadcast recips across partitions via K=33 matmul
                      # (bsel rows other than 0/32 are zero -> garbage killed)
                      bcp = ps_d.tile([128, TQ], F32, tag="db", name="bcp")
                      if p2sub >= 6:
                          nc.tensor.matmul(bcp[:], bselt[:], rd[:],
                                           start=True, stop=True,
                                           skip_group_check=True)
                      if dbg and pair == 0 and tb == 0:
                          nc.sync.dma_start(dbg_t["dbg_yu"][:, :], ysl.bitcast(F32))
                          nc.sync.dma_start(dbg_t["dbg_db"][0:33, :], rd[:].bitcast(F32))
                      # in-place normalize in SBUF (off the PSUM critical path)
                      if p2sub >= 7:
                          nc.vector.tensor_tensor(ysl, ysl, bcp[:], MULT)
                      if dbg and pair == 0 and tb == 0:
                          bstg = nrm.tile([128, TQ], F32, tag="bstg", name="bstg")
                          nc.scalar.activation(bstg[:], bcp[:], COPY)
                          nc.sync.dma_start(dbg_t["dbg_bc"][:, :], bstg[:])
                      if dbg and pair == 0 and tb == 0:
                          nc.sync.dma_start(dbg_t["dbg_y"][:, :],
                                            ysb[0][:, 0, :].bitcast(F32))

        # ---- phase 3: output projection ----------------------------------
        with ExitStack() as ph3:
          if 3 in phases:
              ps_o = ph3.enter_context(tc.tile_pool(name="ps_o", bufs=4, space="PSUM"))
              stg = ph3.enter_context(tc.tile_pool(name="stg", bufs=4))
              for mc in range(c // 128):
                  for nb in range(nt):
                      ps = ps_o.tile([128, TQ], F32, tag="ps_o", name="pso")
                      for pair in range(NPAIR):
                          nc.tensor.matmul(
                              ps[:],
                              wpt[pair][:, mc * 128:(mc + 1) * 128],
                              ysb[pair][:, nb, :],
                              start=(pair == 0), stop=(pair == NPAIR - 1),
                          )
                      ot = stg.tile([128, TQ], F32, tag="ot", name="ot")
                      nc.scalar.activation(ot[:], ps[:], COPY)
                      nc.sync.dma_start(
                          out_t[mc * 128:(mc + 1) * 128, nb * TQ:(nb + 1) * TQ], ot[:])

    nc.compile()
    return nc


def make_in_maps(x, W_attn, W_proj, cache_k, cache_v, n_cores=8):
    """Shard full inputs into per-core input maps (host side)."""
    b_, t_, c_ = x.shape
    h_ = cache_k.shape[1]
    d_ = c_ // h_
    p_ = cache_k.shape[2]
    hpc = h_ // (n_cores // b_)
    in_maps = []
    Wq = W_attn[:, 0 * c_:1 * c_]
    Wk = W_attn[:, 1 * c_:2 * c_]
    Wv = W_attn[:, 2 * c_:3 * c_]
    mask_np = np.zeros((128, 4, TQ), np.float32)
    for j in range(4):
        mask_np[:, j, :] = (np.arange(TQ)[None, :] >=
                            (np.arange(128)[:, None] + j * 128)).astype(np.float32)
    ones_np = np.ones((128, 1), np.float32)
    bsel_np = np.zeros((33, 128), np.float32)
    bsel_np[0, 0:64] = 1.0
    bsel_np[32, 64:128] = 1.0
    for core in range(n_cores):
        b = core // (n_cores // b_)
        h0 = (core % (n_cores // b_)) * hpc
        heads = list(range(h0, h0 + hpc))
        cols = np.concatenate([np.arange(h * d_, (h + 1) * d_) for h in heads])
        x_t = np.ascontiguousarray(x[b].T)                       # [C, T]
        w_qk = np.ascontiguousarray(
            np.concatenate([Wq[:, cols], Wk[:, cols]], axis=1))  # [C, 512]
        w_v = np.ascontiguousarray(Wv[:, cols])                  # [C, 256]
        npair = hpc // 2
        ckt_np = np.zeros((npair, 128, p_), np.float32)
        cv_np = np.zeros((npair, 128, p_ // KC, 128), np.float32)
        wp_np = np.zeros((npair, 128, c_), np.float32)
        for pr in range(npair):
            he, ho = heads[2 * pr], heads[2 * pr + 1]
            ckt_np[pr, 0:64] = cache_k[b, he].T
            ckt_np[pr, 64:128] = cache_k[b, ho].T
            cvr_e = cache_v[b, he].reshape(p_ // KC, KC, d_)     # [chunk, key, d]
            cvr_o = cache_v[b, ho].reshape(p_ // KC, KC, d_)
            cv_np[pr, :, :, 0:64] = cvr_e.transpose(1, 0, 2)
            cv_np[pr, :, :, 64:128] = cvr_o.transpose(1, 0, 2)
            wp_np[pr, 0:64] = W_proj[he * d_:(he + 1) * d_]
            wp_np[pr, 64:128] = W_proj[ho * d_:(ho + 1) * d_]
        in_maps.append({
            "x_t": x_t,
            "w_qk": w_qk,
            "w_v": w_v,
            "ckt": ckt_np,
            "cv": cv_np.astype(ml_dtypes.bfloat16),
            "wp": wp_np,
            "masks": mask_np.astype(ml_dtypes.bfloat16),
            "ones": ones_np.astype(ml_dtypes.bfloat16),
            "bsel": bsel_np,
            "zrd": np.zeros((33, TQ), np.float32),
        })
    return in_maps


def assemble_output(results, n_cores=8, b_=B, t_=T, c_=C):
    """Sum per-core partial out^T over head groups, transpose back."""
    out = np.zeros((b_, t_, c_), np.float32)
    per_b = n_cores // b_
    for b in range(b_):
        acc = np.zeros((c_, t_), np.float32)
        for i in range(per_b):
            acc += results[b * per_b + i]["out_t"]
        out[b] = acc.T
    return out


_NC_CACHE = {}


def kernel(x, W_attn, W_proj, cache_k, cache_v):
    x = np.asarray(x, np.float32)
    W_attn = np.asarray(W_attn, np.float32)
    W_proj = np.asarray(W_proj, np.float32)
    cache_k = np.asarray(cache_k, np.float32)
    cache_v = np.asarray(cache_v, np.float32)
    if "nc" not in _NC_CACHE:
        _NC_CACHE["nc"] = build_kernel()
    nc = _NC_CACHE["nc"]
    in_maps = make_in_maps(x, W_attn, W_proj, cache_k, cache_v)
    res = run_bass_kernel_spmd(nc, in_maps, list(range(8)))
    return assemble_output(res.results)



# revision 19
# speedup vs baseline: 1.1365x; 1.1365x over previous
"""Trainium2 Bass kernel for CausalSelfAttention with KV-prefix cache.

Problem (hardcoded): B=2, T=2048, C=1024, H=16, D=64, P=2048.
Sharding: 8 cores = 2 (batch) x 4 (head groups of 4 heads).
Each core computes, for its (b, 4 heads):
  qkv slice -> prefix+causal softmax -> AV -> partial W_proj product.
Host sums the 4 per-core partial projections per batch and transposes.

All attention math runs in a transposed layout (features/keys on the
partition dim) so no on-device transposes are needed:
  x^T [C,T] -> q^T,k^T (pair-packed [128,T]) via lhsT=W_attn slice
  S^T [keys, tq] via row-tiled (2 heads) K=64 matmuls
  exp on ScalarE (PSUM->SBUF, bf16, fused scale=1/sqrt(D))
  AV per head with an appended ones-column (M=65) so the softmax
  denominator accumulates for free in PSUM partition 64
  per-query normalization via reciprocal + K=65 broadcast matmul
  out^T = W_proj_slice^T @ y^T  (per-core partial)

Phase 2 is software-pipelined: score matmuls for chunk c+1 are issued
before the AV matmuls of chunk c, so the ScalarE exp (the per-chunk
serial dependency) overlaps the TensorE work of neighboring chunks.
"""

import numpy as np
import ml_dtypes
from contextlib import ExitStack

import concourse.bacc as bacc
import concourse.tile as tile
import concourse.mybir as mybir
from concourse.bass_utils import run_bass_kernel_spmd

F32 = mybir.dt.float32
F32R = mybir.dt.float32r
BF16 = mybir.dt.bfloat16
EXP = mybir.ActivationFunctionType.Exp
COPY = mybir.ActivationFunctionType.Copy
MULT = mybir.AluOpType.mult

B, T, C, H, D, P = 2, 2048, 1024, 16, 64, 2048
HPC = 4            # heads per core
NPAIR = 2          # head pairs per core
TQ = 512           # query block (matmul free dim)
KC = 128           # key chunk (PSUM partition dim)
SCALE = 1.0 / np.sqrt(D)


def build_kernel(t=T, p=P, c=C, n_cores=8):
    """Build + compile the SPMD Bass program. Same program on every core."""
    nt = t // TQ            # query blocks
    npc = p // KC           # prefix key chunks
    nck = c // 128          # C contraction chunks
    ntc = t // 128          # T rows in 128-chunks

    nc = bacc.Bacc("TRN2", target_bir_lowering=False, debug=False,
                   num_devices=n_cores)

    x_t = nc.dram_tensor("x_t", [c, t], F32, kind="ExternalInput").ap()
    w_qk = nc.dram_tensor("w_qk", [c, 2 * HPC * D], F32, kind="ExternalInput").ap()
    w_v = nc.dram_tensor("w_v", [c, HPC * D], F32, kind="ExternalInput").ap()
    ckt = nc.dram_tensor("ckt", [NPAIR, 128, p], F32, kind="ExternalInput").ap()
    cv = nc.dram_tensor("cv", [NPAIR, 128, npc, 2, 65], BF16, kind="ExternalInput").ap()
    wp = nc.dram_tensor("wp", [NPAIR, 128, c], F32, kind="ExternalInput").ap()
    masks = nc.dram_tensor("masks", [128, 4, TQ], BF16, kind="ExternalInput").ap()
    bsel = nc.dram_tensor("bsel", [65, 128], F32, kind="ExternalInput").ap()
    zrd = nc.dram_tensor("zrd", [65, 2, TQ], F32, kind="ExternalInput").ap()
    out_t = nc.dram_tensor("out_t", [c, t], F32, kind="ExternalOutput").ap()

    with tile.TileContext(nc) as tc, ExitStack() as top:
        const = top.enter_context(tc.tile_pool(name="const", bufs=1))
        persist = top.enter_context(tc.tile_pool(name="persist", bufs=1))

        # ---- persistent SBUF tensors -------------------------------------
        qT = [persist.tile([128, t], F32R, tag=f"qT{i}", name=f"qT{i}") for i in range(NPAIR)]
        kT = [persist.tile([128, t], F32R, tag=f"kT{i}", name=f"kT{i}") for i in range(NPAIR)]
        cktT = [persist.tile([128, p], F32R, tag=f"cktT{i}", name=f"cktT{i}") for i in range(NPAIR)]
        # v with ones column, [.., eo, pair, 65]: even heads [v|1] (denom at
        # PSUM partition 64), odd heads [1|v] (denom at partition 0)
        vt = persist.tile([128, ntc, 2, NPAIR, 65], BF16, tag="vt", name="vt")
        cvt = [persist.tile([128, npc, 2, 65], BF16, tag=f"cvt{i}", name=f"cvt{i}") for i in range(NPAIR)]
        wpt = [persist.tile([128, c], F32R, tag=f"wpt{i}", name=f"wpt{i}") for i in range(NPAIR)]
        maskt = const.tile([128, 4, TQ], BF16, tag="maskt", name="maskt")
        bselt = const.tile([65, 128], F32R, tag="bselt", name="bselt")
        ysb = [persist.tile([128, nt, TQ], F32R, tag=f"ysb{i}", name=f"ysb{i}") for i in range(NPAIR)]
        # reciprocal rows live at partition 64 (where the AV denom lands);
        # double-buffered along dim 1 so consecutive tb's don't serialize
        rdc = persist.tile([65, 2, TQ], F32R, tag="rdc", name="rdc")

        nc.sync.dma_start(maskt[:], masks[:, :, :])
        nc.sync.dma_start(bselt[:], bsel[:, :].bitcast(F32R))
        nc.sync.dma_start(rdc[:], zrd[:].bitcast(F32R))
        nc.gpsimd.memset(vt[:, :, 0, :, 64:65], 1.0)
        nc.gpsimd.memset(vt[:, :, 1, :, 0:1], 1.0)

        # ---- phase 1: QKV projection (x streamed by query block) ----------
        with ExitStack() as ph1:
            xin = ph1.enter_context(tc.tile_pool(name="xin", bufs=2))
            win = ph1.enter_context(tc.tile_pool(name="win", bufs=1))
            ps_qk = ph1.enter_context(tc.tile_pool(name="ps_qk", bufs=2, space="PSUM"))
            ps_v = ph1.enter_context(tc.tile_pool(name="ps_v", bufs=2, space="PSUM"))

            wqkt = win.tile([128, nck, 2 * HPC * D], F32R, tag="wqkt", name="wqkt")
            wvt = win.tile([128, nck, HPC * D], F32R, tag="wvt", name="wvt")
            for kc_ in range(nck):
                nc.sync.dma_start(wqkt[:, kc_, :], w_qk[kc_ * 128:(kc_ + 1) * 128, :].bitcast(F32R))
                nc.sync.dma_start(wvt[:, kc_, :], w_v[kc_ * 128:(kc_ + 1) * 128, :].bitcast(F32R))

            for nb in range(nt):
                xt = xin.tile([128, nck, TQ], F32R, tag="xt", name="xt")
                for kc_ in range(nck):
                    nc.sync.dma_start(
                        xt[:, kc_, :],
                        x_t[kc_ * 128:(kc_ + 1) * 128, nb * TQ:(nb + 1) * TQ].bitcast(F32R))
                # q^T / k^T: out chunk mc (128 rows = one head pair of q or k)
                for mc in range(4):
                    dest = qT[mc] if mc < 2 else kT[mc - 2]
                    ps = ps_qk.tile([128, TQ], F32, tag="ps_qk", name="psqk")
                    for kc_ in range(nck):
                        nc.tensor.matmul(
                            ps[:],
                            wqkt[:, kc_, mc * 128:(mc + 1) * 128],
                            xt[:, kc_, :],
                            start=(kc_ == 0), stop=(kc_ == nck - 1),
                        )
                    nc.scalar.activation(dest[:, nb * TQ:(nb + 1) * TQ], ps[:], COPY)
                # v in natural layout [t, (eo, pair), 64]; w_v columns are
                # host-ordered evens-first so each evac is one strided copy
                for tcl in range(TQ // 128):
                    tc_ = nb * (TQ // 128) + tcl
                    ps = ps_v.tile([128, 2, NPAIR, D], F32, tag="ps_v", name="psv")
                    for kc_ in range(nck):
                        nc.tensor.matmul(
                            ps[:, :, :, :],
                            xt[:, kc_, tcl * 128:(tcl + 1) * 128],
                            wvt[:, kc_, :],
                            start=(kc_ == 0), stop=(kc_ == nck - 1),
                        )
                    nc.scalar.activation(vt[:, tc_, 0, :, 0:64], ps[:, 0, :, :], COPY)
                    nc.scalar.activation(vt[:, tc_, 1, :, 1:65], ps[:, 1, :, :], COPY)

        # KV-cache + proj weights (needed from phase 2 on; DMA'd behind x)
        for i in range(NPAIR):
            nc.sync.dma_start(cktT[i][:], ckt[i, :, :].bitcast(F32R))
            nc.sync.dma_start(cvt[i][:], cv[i, :, :, :, :])

        # ---- phase 2: attention (software-pipelined) ---------------------
        with ExitStack() as ph2:
            sbp = ph2.enter_context(tc.tile_pool(name="sbp", bufs=2, space="PSUM"))
            ybp = ph2.enter_context(tc.tile_pool(name="ybp", bufs=2, space="PSUM"))
            ebp = ph2.enter_context(tc.tile_pool(name="ebp", bufs=3))
            stp = ph2.enter_context(tc.tile_pool(name="stp", bufs=2))

            units = []
            for pair in range(NPAIR):
                for tb in range(nt):
                    nkc = npc + (tb + 1) * (TQ // KC)
                    for kc_ in range(nkc):
                        units.append((pair, tb, kc_, nkc))

            cur_yb = [None, None]   # [ybe, ybo] accumulators of current tb

            def emit_S(u):
                pair, tb, kc_, nkc = u
                sb = sbp.tile([128, 2, TQ], F32, tag="sb", name="sb")
                for h in range(2):
                    if kc_ < npc:
                        ksrc, klo = cktT[pair], kc_ * KC
                    else:
                        ksrc, klo = kT[pair], (kc_ - npc) * KC
                    nc.tensor.matmul(
                        sb[:, h, :],
                        ksrc[h * 64:(h + 1) * 64, klo:klo + KC],
                        qT[pair][h * 64:(h + 1) * 64, tb * TQ:(tb + 1) * TQ],
                        start=True, stop=True,
                        tile_position=(h * 64, 0),
                        skip_group_check=True,
                    )
                return sb

            def emit_EA(u, sb):
                pair, tb, kc_, nkc = u
                eb = ebp.tile([128, 2, TQ], BF16, tag="eb", name="eb")
                nc.scalar.activation(eb[:, :, :], sb[:, :, :], EXP, scale=SCALE)
                j = kc_ - (nkc - 4)
                if j >= 0:  # causal mask on diagonal chunks
                    for h in range(2):
                        nc.vector.tensor_tensor(
                            eb[:, h, :], eb[:, h, :], maskt[:, j, :], MULT)
                first, last = kc_ == 0, kc_ == nkc - 1
                if first:
                    cur_yb[0] = ybp.tile([65, TQ], F32, tag="ybe", name="ybe")
                    cur_yb[1] = ybp.tile([65, TQ], F32, tag="ybo", name="ybo")
                for h in range(2):
                    if kc_ < npc:
                        vsrc = cvt[pair][:, kc_, h, :]
                    else:
                        vsrc = vt[:, kc_ - npc, h, pair, :]
                    nc.tensor.matmul(
                        cur_yb[h][:, :], vsrc, eb[:, h, :],
                        start=first, stop=last,
                        tile_position=(0, 0),
                        skip_group_check=True,
                    )
                if last:
                    emit_tb_end(pair, tb, cur_yb[0], cur_yb[1])

            def emit_tb_end(pair, tb, ybe, ybo):
                sl = (pair * nt + tb) % 2
                # evacuate y rows; head-odd goes via SBUF->SBUF DMA to reach
                # partitions 64..127 (engines cannot shift partitions)
                nc.vector.tensor_copy(ysb[pair][0:64, tb, :], ybe[0:64, :])
                stage = stp.tile([65, TQ], F32R, tag="stage", name="stage")
                nc.vector.tensor_copy(stage[:, :], ybo[:, :])
                with nc.allow_low_precision(reason="recip->f32r for bcast mm"):
                    nc.vector.reciprocal(rdc[64:65, sl, :], ybe[64:65, :])
                    nc.vector.reciprocal(rdc[0:1, sl, :], ybo[0:1, :])
                nc.sync.dma_start(ysb[pair][64:128, tb, :], stage[1:65, :])
                # broadcast both recips across partitions via one K=65 matmul
                # (bselt rows other than 0/64 are zero -> garbage killed)
                bcpt = sbp.tile([128, 2, TQ], F32, tag="sb", name="bcpt")
                nc.tensor.matmul(bcpt[:, 0, :], bselt[:, :], rdc[:, sl, :],
                                 start=True, stop=True,
                                 skip_group_check=True)
                nc.vector.tensor_tensor(
                    ysb[pair][:, tb, :], ysb[pair][:, tb, :], bcpt[:, 0, :], MULT)

            prev = None
            for u in units:
                sb = emit_S(u)
                if prev is not None:
                    emit_EA(*prev)
                prev = (u, sb)
            emit_EA(*prev)

        nc.sync.dma_start(wpt[0][:], wp[0, :, :].bitcast(F32R))
        nc.sync.dma_start(wpt[1][:], wp[1, :, :].bitcast(F32R))

        # ---- phase 3: output projection ----------------------------------
        with ExitStack() as ph3:
            ps_o = ph3.enter_context(tc.tile_pool(name="ps_o", bufs=4, space="PSUM"))
            stg = ph3.enter_context(tc.tile_pool(name="stg", bufs=4))
            for mc in range(c // 128):
                for nb in range(nt):
                    ps = ps_o.tile([128, TQ], F32, tag="ps_o", name="pso")
                    for pair in range(NPAIR):
                        nc.tensor.matmul(
                            ps[:],
                            wpt[pair][:, mc * 128:(mc + 1) * 128],
                            ysb[pair][:, nb, :],
                            start=(pair == 0), stop=(pair == NPAIR - 1),
                        )
                    ot = stg.tile([128, TQ], F32, tag="ot", name="ot")
                    if (mc * nt + nb) % 2 == 0:
                        nc.scalar.activation(ot[:], ps[:], COPY)
                    else:
                        nc.vector.tensor_copy(ot[:], ps[:])
                    nc.sync.dma_start(
                        out_t[mc * 128:(mc + 1) * 128, nb * TQ:(nb + 1) * TQ], ot[:])

    nc.compile()
    return nc


def make_in_maps(x, W_attn, W_proj, cache_k, cache_v, n_cores=8):
    """Shard full inputs into per-core input maps (host side)."""
    b_, t_, c_ = x.shape
    h_ = cache_k.shape[1]
    d_ = c_ // h_
    p_ = cache_k.shape[2]
    hpc = h_ // (n_cores // b_)
    in_maps = []
    Wq = W_attn[:, 0 * c_:1 * c_]
    Wk = W_attn[:, 1 * c_:2 * c_]
    Wv = W_attn[:, 2 * c_:3 * c_]
    mask_np = np.zeros((128, 4, TQ), np.float32)
    for j in range(4):
        mask_np[:, j, :] = (np.arange(TQ)[None, :] >=
                            (np.arange(128)[:, None] + j * 128)).astype(np.float32)
    bsel_np = np.zeros((65, 128), np.float32)
    bsel_np[64, 0:64] = 1.0     # even-head recip lives at rdc partition 64
    bsel_np[0, 64:128] = 1.0    # odd-head recip lives at rdc partition 0
    for core in range(n_cores):
        b = core // (n_cores // b_)
        h0 = (core % (n_cores // b_)) * hpc
        heads = list(range(h0, h0 + hpc))
        cols = np.concatenate([np.arange(h * d_, (h + 1) * d_) for h in heads])
        x_t = np.ascontiguousarray(x[b].T)                       # [C, T]
        w_qk = np.ascontiguousarray(
            np.concatenate([Wq[:, cols], Wk[:, cols]], axis=1))  # [C, 512]
        # v columns grouped evens-first: [he(pair0), he(pair1), ho(pair0), ho(pair1)]
        vcols = np.concatenate(
            [np.arange(h * d_, (h + 1) * d_) for h in
             [heads[0], heads[2], heads[1], heads[3]]])
        w_v = np.ascontiguousarray(Wv[:, vcols])                 # [C, 256]
        npair = hpc // 2
        npc = p_ // KC
        ckt_np = np.zeros((npair, 128, p_), np.float32)
        cv_np = np.zeros((npair, 128, npc, 2, 65), np.float32)
        wp_np = np.zeros((npair, 128, c_), np.float32)
        for pr in range(npair):
            he, ho = heads[2 * pr], heads[2 * pr + 1]
            ckt_np[pr, 0:64] = cache_k[b, he].T
            ckt_np[pr, 64:128] = cache_k[b, ho].T
            cvr_e = cache_v[b, he].reshape(npc, KC, d_)     # [chunk, key, d]
            cvr_o = cache_v[b, ho].reshape(npc, KC, d_)
            cv_np[pr, :, :, 0, 0:64] = cvr_e.transpose(1, 0, 2)
            cv_np[pr, :, :, 0, 64] = 1.0
            cv_np[pr, :, :, 1, 1:65] = cvr_o.transpose(1, 0, 2)
            cv_np[pr, :, :, 1, 0] = 1.0
            wp_np[pr, 0:64] = W_proj[he * d_:(he + 1) * d_]
            wp_np[pr, 64:128] = W_proj[ho * d_:(ho + 1) * d_]
        in_maps.append({
            "x_t": x_t,
            "w_qk": w_qk,
            "w_v": w_v,
            "ckt": ckt_np,
            "cv": cv_np.astype(ml_dtypes.bfloat16),
            "wp": wp_np,
            "masks": mask_np.astype(ml_dtypes.bfloat16),
            "bsel": bsel_np,
            "zrd": np.zeros((65, 2, TQ), np.float32),
        })
    return in_maps


def assemble_output(results, n_cores=8, b_=B, t_=T, c_=C):
    """Sum per-core partial out^T over head groups, transpose back."""
    out = np.zeros((b_, t_, c_), np.float32)
    per_b = n_cores // b_
    for b in range(b_):
        acc = np.zeros((c_, t_), np.float32)
        for i in range(per_b):
            acc += results[b * per_b + i]["out_t"]
        out[b] = acc.T
    return out


_NC_CACHE = {}


def kernel(x, W_attn, W_proj, cache_k, cache_v):
    x = np.asarray(x, np.float32)
    W_attn = np.asarray(W_attn, np.float32)
    W_proj = np.asarray(W_proj, np.float32)
    cache_k = np.asarray(cache_k, np.float32)
    cache_v = np.asarray(cache_v, np.float32)
    if "nc" not in _NC_CACHE:
        _NC_CACHE["nc"] = build_kernel()
    nc = _NC_CACHE["nc"]
    in_maps = make_in_maps(x, W_attn, W_proj, cache_k, cache_v)
    res = run_bass_kernel_spmd(nc, in_maps, list(range(8)))
    return assemble_output(res.results)


# revision 27
# speedup vs baseline: 1.9840x; 1.7458x over previous
"""Trainium2 Bass kernel for CausalSelfAttention with KV-prefix cache.

Problem (hardcoded): B=2, T=2048, C=1024, H=16, D=64, P=2048.
Sharding: 8 cores = 2 (batch) x 4 (head groups of 4 heads).
Each core computes, for its (b, 4 heads):
  qkv slice -> prefix+causal softmax -> AV -> partial W_proj product.
Host sums the 4 per-core partial projections per batch and transposes.

All attention math runs in a transposed layout (features/keys on the
partition dim) so no on-device transposes are needed:
  x^T [C,T] -> q^T,k^T (pair-packed [128,T]) via lhsT=W_attn slice
  S^T [keys, tq] via row-tiled (2 heads) K=64 matmuls
  exp on ScalarE (PSUM->SBUF, bf16, fused scale=1/sqrt(D))
  AV per head with an appended ones-column (M=65) so the softmax
  denominator accumulates for free in PSUM partition 64
  per-query normalization via reciprocal + K=65 broadcast matmul
  out^T = W_proj_slice^T @ y^T  (per-core partial)

Phase 2 is software-pipelined: score matmuls for chunk c+1 are issued
before the AV matmuls of chunk c, so the ScalarE exp (the per-chunk
serial dependency) overlaps the TensorE work of neighboring chunks.
"""

import numpy as np
import ml_dtypes
from contextlib import ExitStack

import concourse.bacc as bacc
import concourse.tile as tile
import concourse.mybir as mybir
from concourse.bass_utils import run_bass_kernel_spmd

F32 = mybir.dt.float32
F32R = mybir.dt.float32r
BF16 = mybir.dt.bfloat16
EXP = mybir.ActivationFunctionType.Exp
COPY = mybir.ActivationFunctionType.Copy
MULT = mybir.AluOpType.mult

B, T, C, H, D, P = 2, 2048, 1024, 16, 64, 2048
HPC = 4            # heads per core
NPAIR = 2          # head pairs per core
TQ = 512           # query block (matmul free dim)
KC = 128           # key chunk (PSUM partition dim)
SCALE = 1.0 / np.sqrt(D)


def build_kernel(t=T, p=P, c=C, n_cores=8):
    """Build + compile the SPMD Bass program. Same program on every core."""
    nt = t // TQ            # query blocks
    npc = p // KC           # prefix key chunks
    nck = c // 128          # C contraction chunks
    ntc = t // 128          # T rows in 128-chunks

    nc = bacc.Bacc("TRN2", target_bir_lowering=False, debug=False,
                   num_devices=n_cores)

    x_t = nc.dram_tensor("x_t", [c, t], BF16, kind="ExternalInput").ap()
    w_qk = nc.dram_tensor("w_qk", [c, 2 * HPC * D], BF16, kind="ExternalInput").ap()
    w_v = nc.dram_tensor("w_v", [c, HPC * D], BF16, kind="ExternalInput").ap()
    ckt = nc.dram_tensor("ckt", [NPAIR, 128, p], BF16, kind="ExternalInput").ap()
    cv = nc.dram_tensor("cv", [NPAIR, 128, npc, 2, 65], BF16, kind="ExternalInput").ap()
    wp = nc.dram_tensor("wp", [NPAIR, 128, c], BF16, kind="ExternalInput").ap()
    masks = nc.dram_tensor("masks", [128, 4, TQ], BF16, kind="ExternalInput").ap()
    bsel = nc.dram_tensor("bsel", [65, 128], F32, kind="ExternalInput").ap()
    zrd = nc.dram_tensor("zrd", [65, 2 * 4, TQ], F32, kind="ExternalInput").ap()
    out_t = nc.dram_tensor("out_t", [c, t], F32, kind="ExternalOutput").ap()

    with tile.TileContext(nc) as tc, ExitStack() as top:
        const = top.enter_context(tc.tile_pool(name="const", bufs=1))
        persist = top.enter_context(tc.tile_pool(name="persist", bufs=1))

        # ---- persistent SBUF tensors -------------------------------------
        qT = [persist.tile([128, t], BF16, tag=f"qT{i}", name=f"qT{i}") for i in range(NPAIR)]
        kT = [persist.tile([128, t], BF16, tag=f"kT{i}", name=f"kT{i}") for i in range(NPAIR)]
        cktT = [persist.tile([128, p], BF16, tag=f"cktT{i}", name=f"cktT{i}") for i in range(NPAIR)]
        # v with ones column, [.., eo, pair, 65]: even heads [v|1] (denom at
        # PSUM partition 64), odd heads [1|v] (denom at partition 0)
        vt = persist.tile([128, ntc, 2, NPAIR, 65], BF16, tag="vt", name="vt")
        cvt = [persist.tile([128, npc, 2, 65], BF16, tag=f"cvt{i}", name=f"cvt{i}") for i in range(NPAIR)]
        wpt = [persist.tile([128, c], BF16, tag=f"wpt{i}", name=f"wpt{i}") for i in range(NPAIR)]
        maskt = const.tile([128, 4, TQ], BF16, tag="maskt", name="maskt")
        bselt = const.tile([65, 128], F32R, tag="bselt", name="bselt")
        ysb = [persist.tile([128, nt, TQ], BF16, tag=f"ysb{i}", name=f"ysb{i}") for i in range(NPAIR)]
        # reciprocal rows live at partitions 64 (even head) / 0 (odd head);
        # one slot per (pair, tb) since normalization is deferred to phase 3
        rdc = persist.tile([65, NPAIR * nt, TQ], F32R, tag="rdc", name="rdc")

        nc.sync.dma_start(maskt[:], masks[:, :, :])
        nc.sync.dma_start(bselt[:], bsel[:, :].bitcast(F32R))
        nc.sync.dma_start(rdc[:], zrd[:].bitcast(F32R))
        nc.gpsimd.memset(vt[:, :, 0, :, 64:65], 1.0)
        nc.gpsimd.memset(vt[:, :, 1, :, 0:1], 1.0)

        # ---- phase 1: QKV projection (x streamed by query block) ----------
        with ExitStack() as ph1:
            xin = ph1.enter_context(tc.tile_pool(name="xin", bufs=2))
            win = ph1.enter_context(tc.tile_pool(name="win", bufs=1))
            ps_qk = ph1.enter_context(tc.tile_pool(name="ps_qk", bufs=2, space="PSUM"))
            ps_v = ph1.enter_context(tc.tile_pool(name="ps_v", bufs=2, space="PSUM"))

            wqkt = win.tile([128, nck, 2 * HPC * D], BF16, tag="wqkt", name="wqkt")
            wvt = win.tile([128, nck, HPC * D], BF16, tag="wvt", name="wvt")
            for kc_ in range(nck):
                nc.sync.dma_start(wqkt[:, kc_, :], w_qk[kc_ * 128:(kc_ + 1) * 128, :])
                nc.sync.dma_start(wvt[:, kc_, :], w_v[kc_ * 128:(kc_ + 1) * 128, :])

            for nb in range(nt):
                xt = xin.tile([128, nck, TQ], BF16, tag="xt", name="xt")
                for kc_ in range(nck):
                    nc.sync.dma_start(
                        xt[:, kc_, :],
                        x_t[kc_ * 128:(kc_ + 1) * 128, nb * TQ:(nb + 1) * TQ])
                # q^T / k^T: out chunk mc (128 rows = one head pair of q or k)
                for mc in range(4):
                    dest = qT[mc] if mc < 2 else kT[mc - 2]
                    ps = ps_qk.tile([128, TQ], F32, tag="ps_qk", name="psqk")
                    for kc_ in range(nck):
                        nc.tensor.matmul(
                            ps[:],
                            wqkt[:, kc_, mc * 128:(mc + 1) * 128],
                            xt[:, kc_, :],
                            start=(kc_ == 0), stop=(kc_ == nck - 1),
                        )
                    nc.scalar.activation(dest[:, nb * TQ:(nb + 1) * TQ], ps[:], COPY)
                # v in natural layout [t, (eo, pair), 64]; w_v columns are
                # host-ordered evens-first so each evac is one strided copy
                for tcl in range(TQ // 128):
                    tc_ = nb * (TQ // 128) + tcl
                    ps = ps_v.tile([128, 2, NPAIR, D], F32, tag="ps_v", name="psv")
                    for kc_ in range(nck):
                        nc.tensor.matmul(
                            ps[:, :, :, :],
                            xt[:, kc_, tcl * 128:(tcl + 1) * 128],
                            wvt[:, kc_, :],
                            start=(kc_ == 0), stop=(kc_ == nck - 1),
                        )
                    nc.scalar.activation(vt[:, tc_, 0, :, 0:64], ps[:, 0, :, :], COPY)
                    nc.scalar.activation(vt[:, tc_, 1, :, 1:65], ps[:, 1, :, :], COPY)

        # KV-cache (needed from phase 2 on; DMA'd behind x)
        for i in range(NPAIR):
            nc.sync.dma_start(cktT[i][:], ckt[i, :, :])
            nc.sync.dma_start(cvt[i][:], cv[i, :, :, :, :])

        # ---- phase 2: attention (software-pipelined) ---------------------
        with ExitStack() as ph2:
            sbp = ph2.enter_context(tc.tile_pool(name="sbp", bufs=2, space="PSUM"))
            ybp = ph2.enter_context(tc.tile_pool(name="ybp", bufs=2, space="PSUM"))
            ebp = ph2.enter_context(tc.tile_pool(name="ebp", bufs=3))
            stp = ph2.enter_context(tc.tile_pool(name="stp", bufs=2))

            units = []
            for pair in range(NPAIR):
                for tb in range(nt):
                    nkc = npc + (tb + 1) * (TQ // KC)
                    for kc_ in range(nkc):
                        units.append((pair, tb, kc_, nkc))

            cur_yb = [None, None]   # [ybe, ybo] accumulators of current tb

            def emit_S(u):
                pair, tb, kc_, nkc = u
                sb = sbp.tile([128, 2, TQ], F32, tag="sb", name="sb")
                for h in range(2):
                    if kc_ < npc:
                        ksrc, klo = cktT[pair], kc_ * KC
                    else:
                        ksrc, klo = kT[pair], (kc_ - npc) * KC
                    nc.tensor.matmul(
                        sb[:, h, :],
                        ksrc[h * 64:(h + 1) * 64, klo:klo + KC],
                        qT[pair][h * 64:(h + 1) * 64, tb * TQ:(tb + 1) * TQ],
                        start=True, stop=True,
                        tile_position=(h * 64, 0),
                        skip_group_check=True,
                    )
                return sb

            def emit_EA(u, sb):
                pair, tb, kc_, nkc = u
                eb = ebp.tile([128, 2, TQ], BF16, tag="eb", name="eb")
                nc.scalar.activation(eb[:, :, :], sb[:, :, :], EXP, scale=SCALE)
                j = kc_ - (nkc - 4)
                if j >= 0:  # causal mask on diagonal chunks
                    for h in range(2):
                        nc.vector.tensor_tensor(
                            eb[:, h, :], eb[:, h, :], maskt[:, j, :], MULT)
                first, last = kc_ == 0, kc_ == nkc - 1
                if first:
                    cur_yb[0] = ybp.tile([65, TQ], F32, tag="ybe", name="ybe")
                    cur_yb[1] = ybp.tile([65, TQ], F32, tag="ybo", name="ybo")
                for h in range(2):
                    if kc_ < npc:
                        vsrc = cvt[pair][:, kc_, h, :]
                    else:
                        vsrc = vt[:, kc_ - npc, h, pair, :]
                    nc.tensor.matmul(
                        cur_yb[h][:, :], vsrc, eb[:, h, :],
                        start=first, stop=last,
                        tile_position=(0, 0),
                        skip_group_check=True,
                    )
                if last:
                    emit_tb_end(pair, tb, cur_yb[0], cur_yb[1])

            def emit_tb_end(pair, tb, ybe, ybo):
                sl = pair * nt + tb
                # evacuate y rows; head-odd goes via SBUF->SBUF DMA to reach
                # partitions 64..127 (engines cannot shift partitions).
                # Normalization itself is deferred to phase 3.
                nc.vector.tensor_copy(ysb[pair][0:64, tb, :], ybe[0:64, :])
                stage = stp.tile([65, TQ], BF16, tag="stage", name="stage")
                nc.vector.tensor_copy(stage[:, :], ybo[:, :])
                with nc.allow_low_precision(reason="recip->f32r for bcast mm"):
                    nc.vector.reciprocal(rdc[64:65, sl, :], ybe[64:65, :])
                    nc.vector.reciprocal(rdc[0:1, sl, :], ybo[0:1, :])
                nc.sync.dma_start(ysb[pair][64:128, tb, :], stage[1:65, :])

            prev = None
            for u in units:
                sb = emit_S(u)
                if prev is not None:
                    emit_EA(*prev)
                prev = (u, sb)
            emit_EA(*prev)

        nc.sync.dma_start(wpt[0][:], wp[0, :, :])
        nc.sync.dma_start(wpt[1][:], wp[1, :, :])

        # ---- phase 3: softmax normalization + output projection ----------
        with ExitStack() as ph3:
            ps_n = ph3.enter_context(tc.tile_pool(name="ps_n", bufs=2, space="PSUM"))
            ps_o = ph3.enter_context(tc.tile_pool(name="ps_o", bufs=4, space="PSUM"))
            stg = ph3.enter_context(tc.tile_pool(name="stg", bufs=4))
            for nb in range(nt):
                for pair in range(NPAIR):
                    # broadcast both heads' recips across partitions via one
                    # K=65 matmul (bselt rows other than 0/64 are zero)
                    bcp = ps_n.tile([128, TQ], F32, tag="bcp", name="bcp")
                    nc.tensor.matmul(bcp[:], bselt[:, :],
                                     rdc[:, pair * nt + nb, :],
                                     start=True, stop=True,
                                     skip_group_check=True)
                    nc.vector.tensor_tensor(
                        ysb[pair][:, nb, :], ysb[pair][:, nb, :], bcp[:], MULT)
                for mc in range(c // 128):
                    ps = ps_o.tile([128, TQ], F32, tag="ps_o", name="pso")
                    for pair in range(NPAIR):
                        nc.tensor.matmul(
                            ps[:],
                            wpt[pair][:, mc * 128:(mc + 1) * 128],
                            ysb[pair][:, nb, :],
                            start=(pair == 0), stop=(pair == NPAIR - 1),
                        )
                    ot = stg.tile([128, TQ], F32, tag="ot", name="ot")
                    if mc % 2 == 0:
                        nc.scalar.activation(ot[:], ps[:], COPY)
                    else:
                        nc.vector.tensor_copy(ot[:], ps[:])
                    nc.sync.dma_start(
                        out_t[mc * 128:(mc + 1) * 128, nb * TQ:(nb + 1) * TQ], ot[:])

    nc.compile()
    return nc


def make_in_maps(x, W_attn, W_proj, cache_k, cache_v, n_cores=8):
    """Shard full inputs into per-core input maps (host side)."""
    b_, t_, c_ = x.shape
    h_ = cache_k.shape[1]
    d_ = c_ // h_
    p_ = cache_k.shape[2]
    hpc = h_ // (n_cores // b_)
    in_maps = []
    Wq = W_attn[:, 0 * c_:1 * c_]
    Wk = W_attn[:, 1 * c_:2 * c_]
    Wv = W_attn[:, 2 * c_:3 * c_]
    mask_np = np.zeros((128, 4, TQ), np.float32)
    for j in range(4):
        mask_np[:, j, :] = (np.arange(TQ)[None, :] >=
                            (np.arange(128)[:, None] + j * 128)).astype(np.float32)
    bsel_np = np.zeros((65, 128), np.float32)
    bsel_np[64, 0:64] = 1.0     # even-head recip lives at rdc partition 64
    bsel_np[0, 64:128] = 1.0    # odd-head recip lives at rdc partition 0
    for core in range(n_cores):
        b = core // (n_cores // b_)
        h0 = (core % (n_cores // b_)) * hpc
        heads = list(range(h0, h0 + hpc))
        cols = np.concatenate([np.arange(h * d_, (h + 1) * d_) for h in heads])
        x_t = np.ascontiguousarray(x[b].T)                       # [C, T]
        w_qk = np.ascontiguousarray(
            np.concatenate([Wq[:, cols], Wk[:, cols]], axis=1))  # [C, 512]
        # v columns grouped evens-first: [he(pair0), he(pair1), ho(pair0), ho(pair1)]
        vcols = np.concatenate(
            [np.arange(h * d_, (h + 1) * d_) for h in
             [heads[0], heads[2], heads[1], heads[3]]])
        w_v = np.ascontiguousarray(Wv[:, vcols])                 # [C, 256]
        npair = hpc // 2
        npc = p_ // KC
        ckt_np = np.zeros((npair, 128, p_), np.float32)
        cv_np = np.zeros((npair, 128, npc, 2, 65), np.float32)
        wp_np = np.zeros((npair, 128, c_), np.float32)
        for pr in range(npair):
            he, ho = heads[2 * pr], heads[2 * pr + 1]
            ckt_np[pr, 0:64] = cache_k[b, he].T
            ckt_np[pr, 64:128] = cache_k[b, ho].T
            cvr_e = cache_v[b, he].reshape(npc, KC, d_)     # [chunk, key, d]
            cvr_o = cache_v[b, ho].reshape(npc, KC, d_)
            cv_np[pr, :, :, 0, 0:64] = cvr_e.transpose(1, 0, 2)
            cv_np[pr, :, :, 0, 64] = 1.0
            cv_np[pr, :, :, 1, 1:65] = cvr_o.transpose(1, 0, 2)
            cv_np[pr, :, :, 1, 0] = 1.0
            wp_np[pr, 0:64] = W_proj[he * d_:(he + 1) * d_]
            wp_np[pr, 64:128] = W_proj[ho * d_:(ho + 1) * d_]
        in_maps.append({
            "x_t": x_t.astype(ml_dtypes.bfloat16),
            "w_qk": w_qk.astype(ml_dtypes.bfloat16),
            "w_v": w_v.astype(ml_dtypes.bfloat16),
            "ckt": ckt_np.astype(ml_dtypes.bfloat16),
            "cv": cv_np.astype(ml_dtypes.bfloat16),
            "wp": wp_np.astype(ml_dtypes.bfloat16),
            "masks": mask_np.astype(ml_dtypes.bfloat16),
            "bsel": bsel_np,
            "zrd": np.zeros((65, 8, TQ), np.float32),
        })
    return in_maps


def assemble_output(results, n_cores=8, b_=B, t_=T, c_=C):
    """Sum per-core partial out^T over head groups, transpose back."""
    out = np.zeros((b_, t_, c_), np.float32)
    per_b = n_cores // b_
    for b in range(b_):
        acc = np.zeros((c_, t_), np.float32)
        for i in range(per_b):
            acc += results[b * per_b + i]["out_t"]
        out[b] = acc.T
    return out


_NC_CACHE = {}


def kernel(x, W_attn, W_proj, cache_k, cache_v):
    x = np.asarray(x, np.float32)
    W_attn = np.asarray(W_attn, np.float32)
    W_proj = np.asarray(W_proj, np.float32)
    cache_k = np.asarray(cache_k, np.float32)
    cache_v = np.asarray(cache_v, np.float32)
    if "nc" not in _NC_CACHE:
        _NC_CACHE["nc"] = build_kernel()
    nc = _NC_CACHE["nc"]
    in_maps = make_in_maps(x, W_attn, W_proj, cache_k, cache_v)
    res = run_bass_kernel_spmd(nc, in_maps, list(range(8)))
    return assemble_output(res.results)


# revision 32
# speedup vs baseline: 2.0888x; 1.0528x over previous
"""Trainium2 Bass kernel for CausalSelfAttention with KV-prefix cache.

Problem (hardcoded): B=2, T=2048, C=1024, H=16, D=64, P=2048.
Sharding: 8 cores = 2 (batch) x 4 (head groups of 4 heads).
Each core computes, for its (b, 4 heads):
  qkv slice -> prefix+causal softmax -> AV -> partial W_proj product.
Host sums the 4 per-core partial projections per batch and transposes.

All attention math runs in a transposed layout (features/keys on the
partition dim) so no on-device transposes are needed:
  x^T [C,T] -> q^T,k^T (pair-packed [128,T]) via lhsT=W_attn slice
  S^T [keys, tq] via row-tiled (2 heads) K=64 matmuls
  exp on ScalarE (PSUM->SBUF, bf16, fused scale=1/sqrt(D))
  AV per head with an appended ones-column (M=65) so the softmax
  denominator accumulates for free in PSUM partition 64
  per-query normalization via reciprocal + K=65 broadcast matmul
  out^T = W_proj_slice^T @ y^T  (per-core partial)

Phase 2 is software-pipelined: score matmuls for chunk c+1 are issued
before the AV matmuls of chunk c, so the ScalarE exp (the per-chunk
serial dependency) overlaps the TensorE work of neighboring chunks.
"""

import numpy as np
import ml_dtypes
from contextlib import ExitStack

import concourse.bacc as bacc
import concourse.tile as tile
import concourse.mybir as mybir
from concourse.bass_utils import run_bass_kernel_spmd

F32 = mybir.dt.float32
F32R = mybir.dt.float32r
BF16 = mybir.dt.bfloat16
EXP = mybir.ActivationFunctionType.Exp
COPY = mybir.ActivationFunctionType.Copy
MULT = mybir.AluOpType.mult

B, T, C, H, D, P = 2, 2048, 1024, 16, 64, 2048
HPC = 4            # heads per core
NPAIR = 2          # head pairs per core
TQ = 512           # query block (matmul free dim)
KC = 128           # key chunk (PSUM partition dim)
SCALE = 1.0 / np.sqrt(D)


def build_kernel(t=T, p=P, c=C, n_cores=8):
    """Build + compile the SPMD Bass program. Same program on every core."""
    nt = t // TQ            # query blocks
    npc = p // KC           # prefix key chunks
    nck = c // 128          # C contraction chunks
    ntc = t // 128          # T rows in 128-chunks

    nc = bacc.Bacc("TRN2", target_bir_lowering=False, debug=False,
                   num_devices=n_cores)

    x_t = nc.dram_tensor("x_t", [c, t], BF16, kind="ExternalInput").ap()
    w_qk = nc.dram_tensor("w_qk", [c, 2 * HPC * D], BF16, kind="ExternalInput").ap()
    w_v = nc.dram_tensor("w_v", [c, HPC * D], BF16, kind="ExternalInput").ap()
    ckt = nc.dram_tensor("ckt", [NPAIR, 128, p], BF16, kind="ExternalInput").ap()
    cv = nc.dram_tensor("cv", [NPAIR, 128, npc, 2, 65], BF16, kind="ExternalInput").ap()
    wp = nc.dram_tensor("wp", [NPAIR, 128, c], BF16, kind="ExternalInput").ap()
    masks = nc.dram_tensor("masks", [128, 4, TQ], BF16, kind="ExternalInput").ap()
    bsel = nc.dram_tensor("bsel", [65, 128], F32, kind="ExternalInput").ap()
    zrd = nc.dram_tensor("zrd", [65, 2 * 4, TQ], F32, kind="ExternalInput").ap()
    out_t = nc.dram_tensor("out_t", [c, t], BF16, kind="ExternalOutput").ap()

    with tile.TileContext(nc) as tc, ExitStack() as top:
        const = top.enter_context(tc.tile_pool(name="const", bufs=1))
        persist = top.enter_context(tc.tile_pool(name="persist", bufs=1))

        # ---- persistent SBUF tensors -------------------------------------
        qT = [persist.tile([128, t], BF16, tag=f"qT{i}", name=f"qT{i}") for i in range(NPAIR)]
        kT = [persist.tile([128, t], BF16, tag=f"kT{i}", name=f"kT{i}") for i in range(NPAIR)]
        cktT = [persist.tile([128, p], BF16, tag=f"cktT{i}", name=f"cktT{i}") for i in range(NPAIR)]
        # v with ones column, [.., eo, pair, 65]: even heads [v|1] (denom at
        # PSUM partition 64), odd heads [1|v] (denom at partition 0)
        vt = persist.tile([128, ntc, 2, NPAIR, 65], BF16, tag="vt", name="vt")
        cvt = [persist.tile([128, npc, 2, 65], BF16, tag=f"cvt{i}", name=f"cvt{i}") for i in range(NPAIR)]
        wpt = [persist.tile([128, c], BF16, tag=f"wpt{i}", name=f"wpt{i}") for i in range(NPAIR)]
        maskt = const.tile([128, 4, TQ], BF16, tag="maskt", name="maskt")
        bselt = const.tile([65, 128], F32R, tag="bselt", name="bselt")
        ysb = [persist.tile([128, nt, TQ], BF16, tag=f"ysb{i}", name=f"ysb{i}") for i in range(NPAIR)]
        # reciprocal rows live at partitions 64 (even head) / 0 (odd head);
        # one slot per (pair, tb) since normalization is deferred to phase 3
        rdc = persist.tile([65, NPAIR * nt, TQ], F32R, tag="rdc", name="rdc")

        nc.gpsimd.memset(vt[:, :, 0, :, 64:65], 1.0)
        nc.gpsimd.memset(vt[:, :, 1, :, 0:1], 1.0)

        # ---- phase 1: QKV projection (x streamed by query block) ----------
        with ExitStack() as ph1:
            xin = ph1.enter_context(tc.tile_pool(name="xin", bufs=2))
            win = ph1.enter_context(tc.tile_pool(name="win", bufs=1))
            ps_qk = ph1.enter_context(tc.tile_pool(name="ps_qk", bufs=2, space="PSUM"))
            ps_v = ph1.enter_context(tc.tile_pool(name="ps_v", bufs=2, space="PSUM"))

            wqkt = win.tile([128, nck, 2 * HPC * D], BF16, tag="wqkt", name="wqkt")
            wvt = win.tile([128, nck, HPC * D], BF16, tag="wvt", name="wvt")
            # DMA order: get the first qk chain's operands in ASAP
            xt0 = xin.tile([128, nck, TQ], BF16, tag="xt", name="xt")
            for kc_ in range(nck):
                nc.sync.dma_start(wqkt[:, kc_, :], w_qk[kc_ * 128:(kc_ + 1) * 128, :])
                nc.sync.dma_start(xt0[:, kc_, :], x_t[kc_ * 128:(kc_ + 1) * 128, 0:TQ])
            for kc_ in range(nck):
                nc.sync.dma_start(wvt[:, kc_, :], w_v[kc_ * 128:(kc_ + 1) * 128, :])
            nc.sync.dma_start(maskt[:], masks[:, :, :])
            nc.sync.dma_start(bselt[:], bsel[:, :].bitcast(F32R))
            nc.sync.dma_start(rdc[:], zrd[:].bitcast(F32R))

            for nb in range(nt):
                if nb == 0:
                    xt = xt0
                else:
                    xt = xin.tile([128, nck, TQ], BF16, tag="xt", name="xt")
                    for kc_ in range(nck):
                        nc.sync.dma_start(
                            xt[:, kc_, :],
                            x_t[kc_ * 128:(kc_ + 1) * 128, nb * TQ:(nb + 1) * TQ])
                # q^T / k^T: out chunk mc (128 rows = one head pair of q or k)
                for mc in range(4):
                    dest = qT[mc] if mc < 2 else kT[mc - 2]
                    ps = ps_qk.tile([128, TQ], F32, tag="ps_qk", name="psqk")
                    for kc_ in range(nck):
                        nc.tensor.matmul(
                            ps[:],
                            wqkt[:, kc_, mc * 128:(mc + 1) * 128],
                            xt[:, kc_, :],
                            start=(kc_ == 0), stop=(kc_ == nck - 1),
                        )
                    nc.scalar.activation(dest[:, nb * TQ:(nb + 1) * TQ], ps[:], COPY)
                # v in natural layout [t, (eo, pair), 64]; w_v columns are
                # host-ordered evens-first so each evac is one strided copy
                for tcl in range(TQ // 128):
                    tc_ = nb * (TQ // 128) + tcl
                    ps = ps_v.tile([128, 2, NPAIR, D], F32, tag="ps_v", name="psv")
                    for kc_ in range(nck):
                        nc.tensor.matmul(
                            ps[:, :, :, :],
                            xt[:, kc_, tcl * 128:(tcl + 1) * 128],
                            wvt[:, kc_, :],
                            start=(kc_ == 0), stop=(kc_ == nck - 1),
                        )
                    nc.scalar.activation(vt[:, tc_, 0, :, 0:64], ps[:, 0, :, :], COPY)
                    nc.scalar.activation(vt[:, tc_, 1, :, 1:65], ps[:, 1, :, :], COPY)

        # KV-cache (needed from phase 2 on; DMA'd behind x)
        for i in range(NPAIR):
            nc.sync.dma_start(cktT[i][:], ckt[i, :, :])
            nc.sync.dma_start(cvt[i][:], cv[i, :, :, :, :])

        # ---- phase 2: attention (software-pipelined) ---------------------
        with ExitStack() as ph2:
            sbp = ph2.enter_context(tc.tile_pool(name="sbp", bufs=2, space="PSUM"))
            ybp = ph2.enter_context(tc.tile_pool(name="ybp", bufs=2, space="PSUM"))
            ebp = ph2.enter_context(tc.tile_pool(name="ebp", bufs=3))
            stp = ph2.enter_context(tc.tile_pool(name="stp", bufs=2))

            units = []
            for pair in range(NPAIR):
                for tb in range(nt):
                    nkc = npc + (tb + 1) * (TQ // KC)
                    for kc_ in range(nkc):
                        units.append((pair, tb, kc_, nkc))

            cur_yb = [None, None]   # [ybe, ybo] accumulators of current tb

            def emit_S(u):
                pair, tb, kc_, nkc = u
                sb = sbp.tile([128, 2, TQ], F32, tag="sb", name="sb")
                for h in range(2):
                    if kc_ < npc:
                        ksrc, klo = cktT[pair], kc_ * KC
                    else:
                        ksrc, klo = kT[pair], (kc_ - npc) * KC
                    nc.tensor.matmul(
                        sb[:, h, :],
                        ksrc[h * 64:(h + 1) * 64, klo:klo + KC],
                        qT[pair][h * 64:(h + 1) * 64, tb * TQ:(tb + 1) * TQ],
                        start=True, stop=True,
                        tile_position=(h * 64, 0),
                        skip_group_check=True,
                    )
                return sb

            def emit_EA(u, sb):
                pair, tb, kc_, nkc = u
                eb = ebp.tile([128, 2, TQ], BF16, tag="eb", name="eb")
                nc.scalar.activation(eb[:, :, :], sb[:, :, :], EXP, scale=SCALE)
                j = kc_ - (nkc - 4)
                if j >= 0:  # causal mask on diagonal chunks
                    for h in range(2):
                        nc.vector.tensor_tensor(
                            eb[:, h, :], eb[:, h, :], maskt[:, j, :], MULT)
                first, last = kc_ == 0, kc_ == nkc - 1
                if first:
                    cur_yb[0] = ybp.tile([65, TQ], F32, tag="ybe", name="ybe")
                    cur_yb[1] = ybp.tile([65, TQ], F32, tag="ybo", name="ybo")
                for h in range(2):
                    if kc_ < npc:
                        vsrc = cvt[pair][:, kc_, h, :]
                    else:
                        vsrc = vt[:, kc_ - npc, h, pair, :]
                    nc.tensor.matmul(
                        cur_yb[h][:, :], vsrc, eb[:, h, :],
                        start=first, stop=last,
                        tile_position=(0, 0),
                        skip_group_check=True,
                    )
                if last:
                    emit_tb_end(pair, tb, cur_yb[0], cur_yb[1])

            def emit_tb_end(pair, tb, ybe, ybo):
                sl = pair * nt + tb
                # evacuate y rows; head-odd goes via SBUF->SBUF DMA to reach
                # partitions 64..127 (engines cannot shift partitions).
                # Normalization itself is deferred to phase 3.
                nc.vector.tensor_copy(ysb[pair][0:64, tb, :], ybe[0:64, :])
                stage = stp.tile([65, TQ], BF16, tag="stage", name="stage")
                nc.vector.tensor_copy(stage[:, :], ybo[:, :])
                with nc.allow_low_precision(reason="recip->f32r for bcast mm"):
                    nc.vector.reciprocal(rdc[64:65, sl, :], ybe[64:65, :])
                    nc.vector.reciprocal(rdc[0:1, sl, :], ybo[0:1, :])
                nc.sync.dma_start(ysb[pair][64:128, tb, :], stage[1:65, :])

            prev = None
            for u in units:
                sb = emit_S(u)
                if prev is not None:
                    emit_EA(*prev)
                prev = (u, sb)
            emit_EA(*prev)

        nc.sync.dma_start(wpt[0][:], wp[0, :, :])
        nc.sync.dma_start(wpt[1][:], wp[1, :, :])

        # ---- phase 3: softmax normalization + output projection ----------
        with ExitStack() as ph3:
            ps_n = ph3.enter_context(tc.tile_pool(name="ps_n", bufs=2, space="PSUM"))
            ps_o = ph3.enter_context(tc.tile_pool(name="ps_o", bufs=4, space="PSUM"))
            stg = ph3.enter_context(tc.tile_pool(name="stg", bufs=4))

            def normalize(nb):
                for pair in range(NPAIR):
                    # broadcast both heads' recips across partitions via one
                    # K=65 matmul (bselt rows other than 0/64 are zero)
                    bcp = ps_n.tile([128, TQ], F32, tag="bcp", name="bcp")
                    nc.tensor.matmul(bcp[:], bselt[:, :],
                                     rdc[:, pair * nt + nb, :],
                                     start=True, stop=True,
                                     skip_group_check=True)
                    nc.vector.tensor_tensor(
                        ysb[pair][:, nb, :], ysb[pair][:, nb, :], bcp[:], MULT)

            normalize(0)
            for nb in range(nt):
                if nb + 1 < nt:
                    normalize(nb + 1)
                for mc in range(c // 128):
                    ps = ps_o.tile([128, TQ], F32, tag="ps_o", name="pso")
                    for pair in range(NPAIR):
                        nc.tensor.matmul(
                            ps[:],
                            wpt[pair][:, mc * 128:(mc + 1) * 128],
                            ysb[pair][:, nb, :],
                            start=(pair == 0), stop=(pair == NPAIR - 1),
                        )
                    ot = stg.tile([128, TQ], BF16, tag="ot", name="ot")
                    if mc % 2 == 0:
                        nc.scalar.activation(ot[:], ps[:], COPY)
                    else:
                        nc.vector.tensor_copy(ot[:], ps[:])
                    nc.sync.dma_start(
                        out_t[mc * 128:(mc + 1) * 128, nb * TQ:(nb + 1) * TQ], ot[:])

    nc.compile()
    return nc


def make_in_maps(x, W_attn, W_proj, cache_k, cache_v, n_cores=8):
    """Shard full inputs into per-core input maps (host side)."""
    b_, t_, c_ = x.shape
    h_ = cache_k.shape[1]
    d_ = c_ // h_
    p_ = cache_k.shape[2]
    hpc = h_ // (n_cores // b_)
    in_maps = []
    Wq = W_attn[:, 0 * c_:1 * c_]
    Wk = W_attn[:, 1 * c_:2 * c_]
    Wv = W_attn[:, 2 * c_:3 * c_]
    mask_np = np.zeros((128, 4, TQ), np.float32)
    for j in range(4):
        mask_np[:, j, :] = (np.arange(TQ)[None, :] >=
                            (np.arange(128)[:, None] + j * 128)).astype(np.float32)
    bsel_np = np.zeros((65, 128), np.float32)
    bsel_np[64, 0:64] = 1.0     # even-head recip lives at rdc partition 64
    bsel_np[0, 64:128] = 1.0    # odd-head recip lives at rdc partition 0
    for core in range(n_cores):
        b = core // (n_cores // b_)
        h0 = (core % (n_cores // b_)) * hpc
        heads = list(range(h0, h0 + hpc))
        cols = np.concatenate([np.arange(h * d_, (h + 1) * d_) for h in heads])
        x_t = np.ascontiguousarray(x[b].T)                       # [C, T]
        w_qk = np.ascontiguousarray(
            np.concatenate([Wq[:, cols], Wk[:, cols]], axis=1))  # [C, 512]
        # v columns grouped evens-first: [he(pair0), he(pair1), ho(pair0), ho(pair1)]
        vcols = np.concatenate(
            [np.arange(h * d_, (h + 1) * d_) for h in
             [heads[0], heads[2], heads[1], heads[3]]])
        w_v = np.ascontiguousarray(Wv[:, vcols])                 # [C, 256]
        npair = hpc // 2
        npc = p_ // KC
        ckt_np = np.zeros((npair, 128, p_), np.float32)
        cv_np = np.zeros((npair, 128, npc, 2, 65), np.float32)
        wp_np = np.zeros((npair, 128, c_), np.float32)
        for pr in range(npair):
            he, ho = heads[2 * pr], heads[2 * pr + 1]
            ckt_np[pr, 0:64] = cache_k[b, he].T
            ckt_np[pr, 64:128] = cache_k[b, ho].T
            cvr_e = cache_v[b, he].reshape(npc, KC, d_)     # [chunk, key, d]
            cvr_o = cache_v[b, ho].reshape(npc, KC, d_)
            cv_np[pr, :, :, 0, 0:64] = cvr_e.transpose(1, 0, 2)
            cv_np[pr, :, :, 0, 64] = 1.0
            cv_np[pr, :, :, 1, 1:65] = cvr_o.transpose(1, 0, 2)
            cv_np[pr, :, :, 1, 0] = 1.0
            wp_np[pr, 0:64] = W_proj[he * d_:(he + 1) * d_]
            wp_np[pr, 64:128] = W_proj[ho * d_:(ho + 1) * d_]
        in_maps.append({
            "x_t": x_t.astype(ml_dtypes.bfloat16),
            "w_qk": w_qk.astype(ml_dtypes.bfloat16),
            "w_v": w_v.astype(ml_dtypes.bfloat16),
            "ckt": ckt_np.astype(ml_dtypes.bfloat16),
            "cv": cv_np.astype(ml_dtypes.bfloat16),
            "wp": wp_np.astype(ml_dtypes.bfloat16),
            "masks": mask_np.astype(ml_dtypes.bfloat16),
            "bsel": bsel_np,
            "zrd": np.zeros((65, 8, TQ), np.float32),
        })
    return in_maps


def assemble_output(results, n_cores=8, b_=B, t_=T, c_=C):
    """Sum per-core partial out^T over head groups, transpose back."""
    out = np.zeros((b_, t_, c_), np.float32)
    per_b = n_cores // b_
    for b in range(b_):
        acc = np.zeros((c_, t_), np.float32)
        for i in range(per_b):
            acc += np.asarray(results[b * per_b + i]["out_t"], dtype=np.float32)
        out[b] = acc.T
    return out


_NC_CACHE = {}


def kernel(x, W_attn, W_proj, cache_k, cache_v):
    x = np.asarray(x, np.float32)
    W_attn = np.asarray(W_attn, np.float32)
    W_proj = np.asarray(W_proj, np.float32)
    cache_k = np.asarray(cache_k, np.float32)
    cache_v = np.asarray(cache_v, np.float32)
    if "nc" not in _NC_CACHE:
        _NC_CACHE["nc"] = build_kernel()
    nc = _NC_CACHE["nc"]
    in_maps = make_in_maps(x, W_attn, W_proj, cache_k, cache_v)
    res = run_bass_kernel_spmd(nc, in_maps, list(range(8)))
    return assemble_output(res.results)


# revision 34
# speedup vs baseline: 2.1874x; 1.0472x over previous
"""Trainium2 Bass kernel for CausalSelfAttention with KV-prefix cache.

Problem (hardcoded): B=2, T=2048, C=1024, H=16, D=64, P=2048.
Sharding: 8 cores = 2 (batch) x 4 (head groups of 4 heads).
Each core computes, for its (b, 4 heads):
  qkv slice -> prefix+causal softmax -> AV -> partial W_proj product.
Host sums the 4 per-core partial projections per batch and transposes.

All attention math runs in a transposed layout (features/keys on the
partition dim) so no on-device transposes are needed:
  x^T [C,T] -> q^T,k^T (pair-packed [128,T]) via lhsT=W_attn slice
  S^T [keys, tq] via row-tiled (2 heads) K=64 matmuls
  exp on ScalarE (PSUM->SBUF, bf16, fused scale=1/sqrt(D))
  AV per head with an appended ones-column (M=65) so the softmax
  denominator accumulates for free in PSUM partition 64
  per-query normalization via reciprocal + K=65 broadcast matmul
  out^T = W_proj_slice^T @ y^T  (per-core partial)

Phase 2 is software-pipelined: score matmuls for chunk c+1 are issued
before the AV matmuls of chunk c, so the ScalarE exp (the per-chunk
serial dependency) overlaps the TensorE work of neighboring chunks.
"""

import numpy as np
import ml_dtypes
from contextlib import ExitStack

import concourse.bacc as bacc
import concourse.tile as tile
import concourse.mybir as mybir
from concourse.bass_utils import run_bass_kernel_spmd

F32 = mybir.dt.float32
F32R = mybir.dt.float32r
BF16 = mybir.dt.bfloat16
EXP = mybir.ActivationFunctionType.Exp
COPY = mybir.ActivationFunctionType.Copy
MULT = mybir.AluOpType.mult

B, T, C, H, D, P = 2, 2048, 1024, 16, 64, 2048
HPC = 4            # heads per core
NPAIR = 2          # head pairs per core
TQ = 512           # query block (matmul free dim)
KC = 128           # key chunk (PSUM partition dim)
SCALE = 1.0 / np.sqrt(D)


def build_kernel(t=T, p=P, c=C, n_cores=8):
    """Build + compile the SPMD Bass program. Same program on every core."""
    nt = t // TQ            # query blocks
    npc = p // KC           # prefix key chunks
    nck = c // 128          # C contraction chunks
    ntc = t // 128          # T rows in 128-chunks

    nc = bacc.Bacc("TRN2", target_bir_lowering=False, debug=False,
                   num_devices=n_cores)

    x_t = nc.dram_tensor("x_t", [c, t], BF16, kind="ExternalInput").ap()
    w_qk = nc.dram_tensor("w_qk", [c, 2 * HPC * D], BF16, kind="ExternalInput").ap()
    w_v = nc.dram_tensor("w_v", [c, HPC * D], BF16, kind="ExternalInput").ap()
    ckt = nc.dram_tensor("ckt", [NPAIR, 128, p], BF16, kind="ExternalInput").ap()
    cv = nc.dram_tensor("cv", [NPAIR, 128, npc, 2, 65], BF16, kind="ExternalInput").ap()
    wp = nc.dram_tensor("wp", [NPAIR, 128, c], BF16, kind="ExternalInput").ap()
    masks = nc.dram_tensor("masks", [128, 4, TQ], BF16, kind="ExternalInput").ap()
    bsel = nc.dram_tensor("bsel", [65, 128], F32, kind="ExternalInput").ap()
    zrd = nc.dram_tensor("zrd", [65, 2 * 4, TQ], F32, kind="ExternalInput").ap()
    out_t = nc.dram_tensor("out_t", [c, t], BF16, kind="ExternalOutput").ap()

    with tile.TileContext(nc) as tc, ExitStack() as top:
        const = top.enter_context(tc.tile_pool(name="const", bufs=1))
        persist = top.enter_context(tc.tile_pool(name="persist", bufs=1))

        # ---- persistent SBUF tensors -------------------------------------
        qT = [persist.tile([128, t], BF16, tag=f"qT{i}", name=f"qT{i}") for i in range(NPAIR)]
        kT = [persist.tile([128, t], BF16, tag=f"kT{i}", name=f"kT{i}") for i in range(NPAIR)]
        cktT = [persist.tile([128, p], BF16, tag=f"cktT{i}", name=f"cktT{i}") for i in range(NPAIR)]
        # v with ones column, [.., eo, pair, 65]: even heads [v|1] (denom at
        # PSUM partition 64), odd heads [1|v] (denom at partition 0)
        vt = persist.tile([128, ntc, 2, NPAIR, 65], BF16, tag="vt", name="vt")
        cvt = [persist.tile([128, npc, 2, 65], BF16, tag=f"cvt{i}", name=f"cvt{i}") for i in range(NPAIR)]
        wpt = [persist.tile([128, c], BF16, tag=f"wpt{i}", name=f"wpt{i}") for i in range(NPAIR)]
        maskt = const.tile([128, 4, TQ], BF16, tag="maskt", name="maskt")
        bselt = const.tile([65, 128], F32R, tag="bselt", name="bselt")
        ysb = [persist.tile([128, nt, TQ], BF16, tag=f"ysb{i}", name=f"ysb{i}") for i in range(NPAIR)]
        # reciprocal rows live at partitions 64 (even head) / 0 (odd head);
        # one slot per (pair, tb) since normalization is deferred to phase 3
        rdc = persist.tile([65, NPAIR * nt, TQ], F32R, tag="rdc", name="rdc")

        nc.gpsimd.memset(vt[:, :, 0, :, 64:65], 1.0)
        nc.gpsimd.memset(vt[:, :, 1, :, 0:1], 1.0)

        # ---- phase 1: QKV projection (x streamed by query block) ----------
        with ExitStack() as ph1:
            xin = ph1.enter_context(tc.tile_pool(name="xin", bufs=2))
            win = ph1.enter_context(tc.tile_pool(name="win", bufs=1))
            ps_qk = ph1.enter_context(tc.tile_pool(name="ps_qk", bufs=2, space="PSUM"))
            ps_v = ph1.enter_context(tc.tile_pool(name="ps_v", bufs=2, space="PSUM"))

            wqkt = win.tile([128, nck, 2 * HPC * D], BF16, tag="wqkt", name="wqkt")
            wvt = win.tile([128, nck, HPC * D], BF16, tag="wvt", name="wvt")
            # DMA order: get the first qk chain's operands in ASAP
            xt0 = xin.tile([128, nck, TQ], BF16, tag="xt", name="xt")
            for kc_ in range(nck):
                nc.sync.dma_start(wqkt[:, kc_, :], w_qk[kc_ * 128:(kc_ + 1) * 128, :])
                nc.sync.dma_start(xt0[:, kc_, :], x_t[kc_ * 128:(kc_ + 1) * 128, 0:TQ])
            for kc_ in range(nck):
                nc.sync.dma_start(wvt[:, kc_, :], w_v[kc_ * 128:(kc_ + 1) * 128, :])
            nc.sync.dma_start(maskt[:], masks[:, :, :])
            nc.sync.dma_start(bselt[:], bsel[:, :].bitcast(F32R))
            nc.sync.dma_start(rdc[:], zrd[:].bitcast(F32R))

            for nb in range(nt):
                if nb == 0:
                    xt = xt0
                else:
                    xt = xin.tile([128, nck, TQ], BF16, tag="xt", name="xt")
                    for kc_ in range(nck):
                        nc.sync.dma_start(
                            xt[:, kc_, :],
                            x_t[kc_ * 128:(kc_ + 1) * 128, nb * TQ:(nb + 1) * TQ])
                # q^T / k^T: out chunk mc (128 rows = one head pair of q or k)
                for mc in range(4):
                    dest = qT[mc] if mc < 2 else kT[mc - 2]
                    ps = ps_qk.tile([128, TQ], F32, tag="ps_qk", name="psqk")
                    for kc_ in range(nck):
                        nc.tensor.matmul(
                            ps[:],
                            wqkt[:, kc_, mc * 128:(mc + 1) * 128],
                            xt[:, kc_, :],
                            start=(kc_ == 0), stop=(kc_ == nck - 1),
                        )
                    nc.scalar.activation(dest[:, nb * TQ:(nb + 1) * TQ], ps[:], COPY)
                # v in natural layout [t, (eo, pair), 64]; w_v columns are
                # host-ordered evens-first so each evac is one strided copy
                for tcl in range(TQ // 128):
                    tc_ = nb * (TQ // 128) + tcl
                    ps = ps_v.tile([128, 2, NPAIR, D], F32, tag="ps_v", name="psv")
                    for kc_ in range(nck):
                        nc.tensor.matmul(
                            ps[:, :, :, :],
                            xt[:, kc_, tcl * 128:(tcl + 1) * 128],
                            wvt[:, kc_, :],
                            start=(kc_ == 0), stop=(kc_ == nck - 1),
                        )
                    nc.scalar.activation(vt[:, tc_, 0, :, 0:64], ps[:, 0, :, :], COPY)
                    nc.scalar.activation(vt[:, tc_, 1, :, 1:65], ps[:, 1, :, :], COPY)

        # KV-cache (needed from phase 2 on; DMA'd behind x)
        for i in range(NPAIR):
            nc.sync.dma_start(cktT[i][:], ckt[i, :, :])
            nc.sync.dma_start(cvt[i][:], cv[i, :, :, :, :])

        # ---- phase 2: attention (software-pipelined) ---------------------
        with ExitStack() as ph2:
            sbp = ph2.enter_context(tc.tile_pool(name="sbp", bufs=2, space="PSUM"))
            ybp = ph2.enter_context(tc.tile_pool(name="ybp", bufs=2, space="PSUM"))
            ebp = ph2.enter_context(tc.tile_pool(name="ebp", bufs=3))
            stp = ph2.enter_context(tc.tile_pool(name="stp", bufs=2))

            units = []
            for pair in range(NPAIR):
                for tb in range(nt):
                    nkc = npc + (tb + 1) * (TQ // KC)
                    for kc_ in range(nkc):
                        units.append((pair, tb, kc_, nkc))

            cur_yb = [None, None]   # [ybe, ybo] accumulators of current tb

            def emit_S(u):
                pair, tb, kc_, nkc = u
                sb = sbp.tile([128, 2, TQ], F32, tag="sb", name="sb")
                for h in range(2):
                    if kc_ < npc:
                        ksrc, klo = cktT[pair], kc_ * KC
                    else:
                        ksrc, klo = kT[pair], (kc_ - npc) * KC
                    nc.tensor.matmul(
                        sb[:, h, :],
                        ksrc[h * 64:(h + 1) * 64, klo:klo + KC],
                        qT[pair][h * 64:(h + 1) * 64, tb * TQ:(tb + 1) * TQ],
                        start=True, stop=True,
                        tile_position=(h * 64, 0),
                        skip_group_check=True,
                    )
                return sb

            def emit_E(u, sb):
                pair, tb, kc_, nkc = u
                eb = ebp.tile([128, 2, TQ], BF16, tag="eb", name="eb")
                nc.scalar.activation(eb[:, :, :], sb[:, :, :], EXP, scale=SCALE)
                j = kc_ - (nkc - 4)
                if j >= 0:  # causal mask on diagonal chunks
                    for h in range(2):
                        nc.vector.tensor_tensor(
                            eb[:, h, :], eb[:, h, :], maskt[:, j, :], MULT)
                return eb

            def emit_A(u, eb):
                pair, tb, kc_, nkc = u
                first, last = kc_ == 0, kc_ == nkc - 1
                if first:
                    cur_yb[0] = ybp.tile([65, TQ], F32, tag="ybe", name="ybe")
                    cur_yb[1] = ybp.tile([65, TQ], F32, tag="ybo", name="ybo")
                for h in range(2):
                    if kc_ < npc:
                        vsrc = cvt[pair][:, kc_, h, :]
                    else:
                        vsrc = vt[:, kc_ - npc, h, pair, :]
                    nc.tensor.matmul(
                        cur_yb[h][:, :], vsrc, eb[:, h, :],
                        start=first, stop=last,
                        tile_position=(0, 0),
                        skip_group_check=True,
                    )
                if last:
                    emit_tb_end(pair, tb, cur_yb[0], cur_yb[1])

            def emit_tb_end(pair, tb, ybe, ybo):
                sl = pair * nt + tb
                # evacuate y rows; head-odd goes via SBUF->SBUF DMA to reach
                # partitions 64..127 (engines cannot shift partitions).
                # Normalization itself is deferred to phase 3.
                nc.vector.tensor_copy(ysb[pair][0:64, tb, :], ybe[0:64, :])
                stage = stp.tile([65, TQ], BF16, tag="stage", name="stage")
                nc.vector.tensor_copy(stage[:, :], ybo[:, :])
                with nc.allow_low_precision(reason="recip->f32r for bcast mm"):
                    nc.vector.reciprocal(rdc[64:65, sl, :], ybe[64:65, :])
                    nc.vector.reciprocal(rdc[0:1, sl, :], ybo[0:1, :])
                nc.sync.dma_start(ysb[pair][64:128, tb, :], stage[1:65, :])

            # Two-stage software pipeline: AV lags exp by one full chunk so
            # the exp->AV semaphore is already satisfied when PE reaches the
            # AV matmuls (no per-chunk PE stall -> PE clock stays ramped).
            pend_e = None   # (unit, sb) awaiting exp
            pend_a = None   # (unit, eb) awaiting AV
            for u in units:
                sb = emit_S(u)
                if pend_a is not None:
                    emit_A(*pend_a)
                    pend_a = None
                if pend_e is not None:
                    pend_a = (pend_e[0], emit_E(*pend_e))
                pend_e = (u, sb)
            pend_a_last = (pend_e[0], emit_E(*pend_e))
            if pend_a is not None:
                emit_A(*pend_a)
            emit_A(*pend_a_last)

        nc.sync.dma_start(wpt[0][:], wp[0, :, :])
        nc.sync.dma_start(wpt[1][:], wp[1, :, :])

        # ---- phase 3: softmax normalization + output projection ----------
        with ExitStack() as ph3:
            ps_n = ph3.enter_context(tc.tile_pool(name="ps_n", bufs=2, space="PSUM"))
            ps_o = ph3.enter_context(tc.tile_pool(name="ps_o", bufs=4, space="PSUM"))
            stg = ph3.enter_context(tc.tile_pool(name="stg", bufs=4))

            def normalize(nb):
                for pair in range(NPAIR):
                    # broadcast both heads' recips across partitions via one
                    # K=65 matmul (bselt rows other than 0/64 are zero)
                    bcp = ps_n.tile([128, TQ], F32, tag="bcp", name="bcp")
                    nc.tensor.matmul(bcp[:], bselt[:, :],
                                     rdc[:, pair * nt + nb, :],
                                     start=True, stop=True,
                                     skip_group_check=True)
                    nc.vector.tensor_tensor(
                        ysb[pair][:, nb, :], ysb[pair][:, nb, :], bcp[:], MULT)

            normalize(0)
            for nb in range(nt):
                if nb + 1 < nt:
                    normalize(nb + 1)
                for mc in range(c // 128):
                    ps = ps_o.tile([128, TQ], F32, tag="ps_o", name="pso")
                    for pair in range(NPAIR):
                        nc.tensor.matmul(
                            ps[:],
                            wpt[pair][:, mc * 128:(mc + 1) * 128],
                            ysb[pair][:, nb, :],
                            start=(pair == 0), stop=(pair == NPAIR - 1),
                        )
                    ot = stg.tile([128, TQ], BF16, tag="ot", name="ot")
                    if mc % 2 == 0:
                        nc.scalar.activation(ot[:], ps[:], COPY)
                    else:
                        nc.vector.tensor_copy(ot[:], ps[:])
                    nc.sync.dma_start(
                        out_t[mc * 128:(mc + 1) * 128, nb * TQ:(nb + 1) * TQ], ot[:])

    nc.compile()
    return nc


def make_in_maps(x, W_attn, W_proj, cache_k, cache_v, n_cores=8):
    """Shard full inputs into per-core input maps (host side)."""
    b_, t_, c_ = x.shape
    h_ = cache_k.shape[1]
    d_ = c_ // h_
    p_ = cache_k.shape[2]
    hpc = h_ // (n_cores // b_)
    in_maps = []
    Wq = W_attn[:, 0 * c_:1 * c_]
    Wk = W_attn[:, 1 * c_:2 * c_]
    Wv = W_attn[:, 2 * c_:3 * c_]
    mask_np = np.zeros((128, 4, TQ), np.float32)
    for j in range(4):
        mask_np[:, j, :] = (np.arange(TQ)[None, :] >=
                            (np.arange(128)[:, None] + j * 128)).astype(np.float32)
    bsel_np = np.zeros((65, 128), np.float32)
    bsel_np[64, 0:64] = 1.0     # even-head recip lives at rdc partition 64
    bsel_np[0, 64:128] = 1.0    # odd-head recip lives at rdc partition 0
    for core in range(n_cores):
        b = core // (n_cores // b_)
        h0 = (core % (n_cores // b_)) * hpc
        heads = list(range(h0, h0 + hpc))
        cols = np.concatenate([np.arange(h * d_, (h + 1) * d_) for h in heads])
        x_t = np.ascontiguousarray(x[b].T)                       # [C, T]
        w_qk = np.ascontiguousarray(
            np.concatenate([Wq[:, cols], Wk[:, cols]], axis=1))  # [C, 512]
        # v columns grouped evens-first: [he(pair0), he(pair1), ho(pair0), ho(pair1)]
        vcols = np.concatenate(
            [np.arange(h * d_, (h + 1) * d_) for h in
             [heads[0], heads[2], heads[1], heads[3]]])
        w_v = np.ascontiguousarray(Wv[:, vcols])                 # [C, 256]
        npair = hpc // 2
        npc = p_ // KC
        ckt_np = np.zeros((npair, 128, p_), np.float32)
        cv_np = np.zeros((npair, 128, npc, 2, 65), np.float32)
        wp_np = np.zeros((npair, 128, c_), np.float32)
        for pr in range(npair):
            he, ho = heads[2 * pr], heads[2 * pr + 1]
            ckt_np[pr, 0:64] = cache_k[b, he].T
            ckt_np[pr, 64:128] = cache_k[b, ho].T
            cvr_e = cache_v[b, he].reshape(npc, KC, d_)     # [chunk, key, d]
            cvr_o = cache_v[b, ho].reshape(npc, KC, d_)
            cv_np[pr, :, :, 0, 0:64] = cvr_e.transpose(1, 0, 2)
            cv_np[pr, :, :, 0, 64] = 1.0
            cv_np[pr, :, :, 1, 1:65] = cvr_o.transpose(1, 0, 2)
            cv_np[pr, :, :, 1, 0] = 1.0
            wp_np[pr, 0:64] = W_proj[he * d_:(he + 1) * d_]
            wp_np[pr, 64:128] = W_proj[ho * d_:(ho + 1) * d_]
        in_maps.append({
            "x_t": x_t.astype(ml_dtypes.bfloat16),
            "w_qk": w_qk.astype(ml_dtypes.bfloat16),
            "w_v": w_v.astype(ml_dtypes.bfloat16),
            "ckt": ckt_np.astype(ml_dtypes.bfloat16),
            "cv": cv_np.astype(ml_dtypes.bfloat16),
            "wp": wp_np.astype(ml_dtypes.bfloat16),
            "masks": mask_np.astype(ml_dtypes.bfloat16),
            "bsel": bsel_np,
            "zrd": np.zeros((65, 8, TQ), np.float32),
        })
    return in_maps


def assemble_output(results, n_cores=8, b_=B, t_=T, c_=C):
    """Sum per-core partial out^T over head groups, transpose back."""
    out = np.zeros((b_, t_, c_), np.float32)
    per_b = n_cores // b_
    for b in range(b_):
        acc = np.zeros((c_, t_), np.float32)
        for i in range(per_b):
            acc += np.asarray(results[b * per_b + i]["out_t"], dtype=np.float32)
        out[b] = acc.T
    return out


_NC_CACHE = {}


def kernel(x, W_attn, W_proj, cache_k, cache_v):
    x = np.asarray(x, np.float32)
    W_attn = np.asarray(W_attn, np.float32)
    W_proj = np.asarray(W_proj, np.float32)
    cache_k = np.asarray(cache_k, np.float32)
    cache_v = np.asarray(cache_v, np.float32)
    if "nc" not in _NC_CACHE:
        _NC_CACHE["nc"] = build_kernel()
    nc = _NC_CACHE["nc"]
    in_maps = make_in_maps(x, W_attn, W_proj, cache_k, cache_v)
    res = run_bass_kernel_spmd(nc, in_maps, list(range(8)))
    return assemble_output(res.results)
